# revision 12
# baseline (speedup 1.0000x reference)
"""Spikformer-style block (conv1x1+BN+LIF, policy-masked spiking attention, MLP)
on 8 Trainium2 NeuronCores, data-parallel over batch.

Each core handles 4 batches, processed as 2 passes of 2 batches (SBUF budget).
Layout: activations [C-chunk(128), (b, n)] with 2 batches, n=196 -> F=392.
All arithmetic fp32. Attention uses the exact decomposition
  o[n] = q[n] @ (k^T diag(pr) v) + (1-pr[n]) (q[n].k[n]) v[n]
which is integer-exact for binary spikes/policy.
LIF state is stored negated (W = -v_reset, or -2*v_reset for sign-coded layers)
so each step is: v = (W * leak) + psum ; spike ; W' = (spike - 1) * v.
"""

from contextlib import ExitStack

import numpy as np

import concourse.bass as bass
from concourse import mybir, tile
from concourse.mybir import AluOpType as Op
from concourse.bass_utils import run_bass_kernel_spmd

T, B, C, N, H, D, HID = 4, 32, 384, 196, 12, 32, 1536
EPS = 1e-5
NCORES = 8
NB = B // NCORES          # batches per core (4)
NBP = 2                   # batches per pass
NPASS = NB // NBP
F = NBP * N               # 392 free elements per tile row
NCH = C // 128            # 3 channel chunks
NH1 = HID // 128          # 12 hidden chunks
DT = mybir.dt.float32

_CACHED = {}


def _split_multiwaits(nc):
    """Hardware TPB instructions hold one sync wait; hoist extras onto
    injected same-engine NoOps placed immediately before."""
    ctr = 0
    for f in nc.m.functions:
        for blk in f.blocks:
            insts = blk.instructions
            new = []
            changed = False
            for inst in insts:
                si = inst.sync_info
                if si is not None and si.on_wait and len(si.on_wait) > 1:
                    waits = list(si.on_wait)
                    for w in waits[:-1]:
                        ctr += 1
                        nop = mybir.InstNoOp(name=f"I-wsplit{ctr}")
                        nop.engine = inst.engine
                        nop.sync_info = mybir.SyncInfo(on_wait=[w], on_update=[])
                        new.append(nop)
                    inst.sync_info = mybir.SyncInfo(
                        on_wait=[waits[-1]], on_update=list(si.on_update or []))
                    changed = True
                new.append(inst)
            if changed:
                blk.instructions = new


def _fold(w, b, g, beta, m, var):
    """0.5*conv_bn(x) = W_eff @ x + d."""
    inv = g / np.sqrt(var + EPS)
    shift = (b if b is not None else 0.0) * inv + beta - m * inv
    return 0.5 * inv[:, None] * w, 0.5 * shift


def _prep_host(inputs):
    f32 = lambda a: np.ascontiguousarray(a, dtype=np.float32)
    x = np.asarray(inputs["x"], dtype=np.float32)
    pol = np.asarray(inputs["policy"], dtype=np.float32).reshape(T, B, N)

    wq, dq = _fold(inputs["wq"], None, inputs["qg"], inputs["qb"], inputs["qm"], inputs["qv"])
    wk, dk = _fold(inputs["wk"], None, inputs["kg"], inputs["kb"], inputs["km"], inputs["kvv"])
    wv, dv = _fold(inputs["wv"], None, inputs["vg"], inputs["vb"], inputs["vm"], inputs["vvv"])
    wp, dp = _fold(inputs["wp"], inputs["bp"], inputs["pg"], inputs["pb"], inputs["pm"], inputs["pv"])
    w1, d1 = _fold(inputs["w1"], inputs["b1"], inputs["g1"], inputs["be1"], inputs["m1"], inputs["v1"])
    w2, d2 = _fold(inputs["w2"], inputs["b2"], inputs["g2"], inputs["be2"], inputs["m2"], inputs["v2"])

    # encodings: q/k/v consume x+1; proj/fc2 consume +-1 spikes; fc1 consumes xr+0.5
    dq = dq - wq.sum(1); dk = dk - wk.sum(1); dv = dv - wv.sum(1)
    dp = dp + 0.5 * wp.sum(1); wp = 0.5 * wp
    d1 = d1 - 0.5 * w1.sum(1)
    d2 = d2 + 0.5 * w2.sum(1); w2 = 0.5 * w2

    def wT(w):  # [O,I] -> [I//128, 128, O]
        I = w.shape[1]
        return f32(w.T.reshape(I // 128, 128, -1))

    # pack big [128, *] constants into one wall, [1, *] rows into another
    wallw = np.concatenate(
        [np.concatenate(list(wT(w)), axis=1) for w in (wq, wk, wv, wp, w1, w2)]
        + [np.eye(128, dtype=np.float32),
           np.kron(np.eye(4), np.ones((32, 32))).astype(np.float32)], axis=1)
    walld = np.concatenate(
        [dq[None], dk[None], dv[None], dp[None], d1[None], d2[None],
         np.ones((1, F), np.float32), np.ones((1, 128), np.float32)], axis=1)
    com = {"wallw": f32(wallw), "walld": f32(walld)}

    xp1 = f32((x + 1.0).reshape(T, B, NCH, 128, N).transpose(0, 2, 3, 1, 4))
    prm1 = f32(1.0 - pol)

    in_maps = []
    for c in range(NCORES):
        bs = slice(c * NB, (c + 1) * NB)
        m = dict(com)
        m["xp1"] = f32(xp1[:, :, :, bs, :])
        m["prcol"] = f32(pol[:, bs, :].reshape(T, NB, N, 1))
        m["prm1row"] = f32(prm1[:, bs, :].reshape(T, 1, NB * N))
        in_maps.append(m)
    return in_maps


def _build_program():
    nc = bass.Bass("TRN2")
    p = lambda name, shape: nc.declare_dram_parameter(name, list(shape), DT, isOutput=False)

    xp1_d = p("xp1", (T, NCH, 128, NB, N))
    WTW = 3 * C + 3 * C + 3 * C + 3 * C + 3 * HID + 12 * C + 256
    WTD = 4 * C + HID + C + F + 128
    wallw_d = p("wallw", (128, WTW))
    walld_d = p("walld", (1, WTD))
    prcol_d = p("prcol", (T, NB, N, 1))
    prm1_d = p("prm1row", (T, 1, NB * N))
    out_d = nc.declare_dram_parameter("out", [T, NCH, 128, NB, N], DT, isOutput=True)

    SIGN = mybir.ActivationFunctionType.Sign
    COPY = mybir.ActivationFunctionType.Copy

    with tile.TileContext(nc) as tc, ExitStack() as ctx:
        cst = ctx.enter_context(tc.tile_pool(name="cst", bufs=1))
        st = ctx.enter_context(tc.tile_pool(name="state", bufs=1))
        act = ctx.enter_context(tc.tile_pool(name="act", bufs=2))
        sp = ctx.enter_context(tc.tile_pool(name="spk", bufs=1))
        scr = ctx.enter_context(tc.tile_pool(name="scr", bufs=2))
        ps = ctx.enter_context(tc.tile_pool(name="ps", bufs=2, space="PSUM"))

        dma = nc.gpsimd.dma_start

        # ---- constants / weights: two wall DMAs, sliced views ----
        wallw = cst.tile([128, WTW], DT, name="wallw", tag="wallw")
        dma(wallw[:], wallw_d[:])
        walld = cst.tile([1, WTD], DT, name="walld", tag="walld")
        dma(walld[:], walld_d[:])
        bias_nh = cst.tile([128, 1], DT, name="bias_nh", tag="bias_nh")
        nc.vector.memset(bias_nh[:], -0.5)
        bias_n1 = cst.tile([128, 1], DT, name="bias_n1", tag="bias_n1")
        nc.vector.memset(bias_n1[:], -1.0)
        wsb, dsb = {}, {}
        off = 0
        for L, nci in (("q", NCH), ("k", NCH), ("v", NCH), ("p", NCH), ("1", NCH), ("2", NH1)):
            O = HID if L == "1" else C
            wsb[L] = []
            for ci in range(nci):
                wsb[L].append(wallw[:, off:off + O])
                off += O
        ident = wallw[:, off:off + 128]; off += 128
        bones = wallw[:, off:off + 128]; off += 128
        doff = 0
        for L in ("q", "k", "v", "p", "1", "2"):
            O = HID if L == "1" else C
            dsb[L] = walld[:, doff:doff + O]
            doff += O
        onesr = walld[:, doff:doff + F]; doff += F
        onesc = walld[:, doff:doff + 128]; doff += 128

        # ---- persistent LIF states (re-zeroed per pass) ----
        def states(tag, n):
            return [st.tile([128, F], DT, name=f"{tag}{i}", tag=f"{tag}{i}") for i in range(n)]

        MSB = [[st.tile([128, 128], DT, name=f"MSB{r}{j}", tag=f"MSB{r}{j}")
                for j in range(NCH)] for r in range(2)]
        for r in range(2):
            for j in range(NCH):
                nc.vector.memset(MSB[r][j][:], 0.0)
        Wq, Wk, Wv = states("Wq", NCH), states("Wk", NCH), states("Wv", NCH)
        Wa, Wpj = states("Wa", NCH), states("Wp", NCH)
        W1, W2 = states("W1", NH1), states("W2", NCH)
        ALL_W = Wq + Wk + Wv + Wa + Wpj + W1 + W2

        def conv(wL, rhs_tiles, nci, co, P):
            for ci in range(nci):
                nc.tensor.matmul(
                    P[:, 0:F], lhsT=wsb[wL][ci][:, co * 128:(co + 1) * 128],
                    rhs=rhs_tiles[ci][:, 0:F], start=(ci == 0), stop=False)
            nc.tensor.matmul(
                P[:, 0:F], lhsT=dsb[wL][0:1, co * 128:(co + 1) * 128],
                rhs=onesr[0:1, 0:F], start=False, stop=True)

        def lif_add(P, Wt, vtmp, leak):
            nc.vector.scalar_tensor_tensor(
                vtmp[:, :], Wt[:, :], leak, P[:, 0:F], Op.mult, Op.add)

        def reset(vtmp, S, Wt):
            nc.vector.scalar_tensor_tensor(
                Wt[:, :], S[:, :], 1.0, vtmp[:, :], Op.subtract, Op.mult)

        # =========================== pass / time loops ===========================
        for pp in range(NPASS):
            bo = pp * NBP  # batch offset within this core's 4
            for Wt in ALL_W:
                nc.vector.memset(Wt[:], 0.0)

            for t in range(T):
                xp1 = [act.tile([128, F], DT, name=f"xp1{j}", tag=f"xp1{j}") for j in range(NCH)]
                for j in range(NCH):
                    dma(xp1[j][:], xp1_d[t, j, :, bo:bo + NBP, :].rearrange("p b n -> p (b n)"))

                # ---- q,k,v convs + LIF ({0,1} spikes) ----
                Sq = [sp.tile([128, F], DT, name=f"Sq{j}", tag=f"Sq{j}") for j in range(NCH)]
                Sk = [sp.tile([128, F], DT, name=f"Sk{j}", tag=f"Sk{j}") for j in range(NCH)]
                Sv = [sp.tile([128, F], DT, name=f"Sv{j}", tag=f"Sv{j}") for j in range(NCH)]
                for L, Wx, Sx in (("q", Wq, Sq), ("k", Wk, Sk), ("v", Wv, Sv)):
                    for co in range(NCH):
                        P = ps.tile([128, F], DT, name="Pcv", tag="Pcv")
                        conv(L, xp1, NCH, co, P)
                        vtmp = scr.tile([128, F], DT, name="vtmp", tag="vtmp")
                        lif_add(P, Wx[co], vtmp, -0.5)
                        nc.vector.tensor_scalar(Sx[co][:, :], vtmp[:, :], 1.0, None, Op.is_ge)
                        reset(vtmp, Sx[co], Wx[co])

                # ---- attention ----
                prm1_r = scr.tile([1, F], DT, name="prm1r", tag="prm1r", bufs=1)
                dma(prm1_r[:], prm1_d[t, :, bo * N:(bo + NBP) * N])
                Pb = ps.tile([128, F], DT, name="Pcv", tag="Pcv")
                nc.tensor.matmul(Pb[:, 0:F], lhsT=onesc[0:1, :], rhs=prm1_r[0:1, 0:F],
                                 start=True, stop=True)
                prm1_bc = scr.tile([128, F], DT, name="prm1bc", tag="prm1bc", bufs=1)
                nc.scalar.activation(prm1_bc[:, :], Pb[:, 0:F], COPY)

                o_sb = [scr.tile([128, F], DT, name=f"osb{j}", tag=f"osb{j}", bufs=1)
                        for j in range(NCH)]
                for b in range(NBP):
                    # transposes of Sk, Sv -> [n, c] layout; policy folded into v
                    KT, VT = [], []
                    for ns in range(2):
                        w_ = 128 if ns == 0 else N - 128
                        pc = scr.tile([w_, 1], DT, name=f"prc{ns}", tag=f"prc{ns}")
                        dma(pc[:], prcol_d[t, bo + b, ns * 128: ns * 128 + w_])
                        kt = scr.tile([w_, C], DT, name=f"KT{ns}", tag=f"KT{ns}")
                        vt = scr.tile([w_, C], DT, name=f"VT{ns}", tag=f"VT{ns}")
                        for src, dst, scale in ((Sk, kt, None), (Sv, vt, pc)):
                            for j in range(NCH):
                                Pt = ps.tile([128, 128], DT, name="Ptp", tag="Ptp")
                                nc.tensor.transpose(
                                    Pt[0:w_, 0:128],
                                    src[j][:, b * N + ns * 128: b * N + ns * 128 + w_],
                                    ident[:, :])
                                if scale is None:
                                    nc.scalar.activation(dst[:, j * 128:(j + 1) * 128],
                                                         Pt[0:w_, 0:128], COPY)
                                else:
                                    nc.scalar.activation(dst[:, j * 128:(j + 1) * 128],
                                                         Pt[0:w_, 0:128], COPY,
                                                         scale=scale[:, 0:1])
                        KT.append(kt); VT.append(vt)
                    # per-chunk cross-head Gram; keep only diagonal 32x32 blocks
                    # (= per-head M, 0.125 folded on evict into pre-zeroed
                    # block-diagonal tiles), then one full-K matmul applies it.
                    for j in range(NCH):
                        Pg = ps.tile([128, 128], DT, name="Pg", tag="Ptp")
                        for ns in range(2):
                            nc.tensor.matmul(
                                Pg[:, 0:128],
                                lhsT=KT[ns][:, j * 128:(j + 1) * 128],
                                rhs=VT[ns][:, j * 128:(j + 1) * 128],
                                start=(ns == 0), stop=(ns == 1))
                        mb = MSB[b % 2][j]
                        for hl in range(4):
                            nc.scalar.activation(
                                mb[32 * hl:32 * hl + 32, 32 * hl:32 * hl + 32],
                                Pg[32 * hl:32 * hl + 32, 32 * hl:32 * hl + 32],
                                COPY, scale=0.125)
                        Po = ps.tile([128, N], DT, name="Po", tag="Po")
                        nc.tensor.matmul(Po[:, 0:N], lhsT=mb[:, :],
                                         rhs=Sq[j][:, b * N:(b + 1) * N],
                                         start=True, stop=True)
                        nc.scalar.activation(o_sb[j][:, b * N:(b + 1) * N], Po[:, 0:N], COPY)

                # diagonal correction: o += 0.125 * (1-pr) * (q.k)_head * v
                for j in range(NCH):
                    qk = scr.tile([128, F], DT, name="qk", tag="qk", bufs=1)
                    nc.vector.tensor_tensor(qk[:, :], Sq[j][:, :], Sk[j][:, :], Op.mult)
                    Pc = ps.tile([128, F], DT, name="Pcv", tag="Pcv")
                    nc.tensor.matmul(Pc[:, 0:F], lhsT=bones[:, :], rhs=qk[:, 0:F],
                                     start=True, stop=True)
                    wti = scr.tile([128, F], DT, name="wti", tag="wti", bufs=1)
                    nc.vector.tensor_tensor(wti[:, :], Pc[:, 0:F], prm1_bc[:, :], Op.mult)
                    u = scr.tile([128, F], DT, name="u", tag="u", bufs=1)
                    nc.vector.scalar_tensor_tensor(u[:, :], wti[:, :], 0.125, Sv[j][:, :],
                                                   Op.mult, Op.mult)
                    nc.vector.tensor_tensor(o_sb[j][:, :], o_sb[j][:, :], u[:, :], Op.add)

                # ---- attn LIF (thr 0.5, sign-coded) ----
                Ea = [sp.tile([128, F], DT, name=f"Ea{j}", tag=f"Ea{j}") for j in range(NCH)]
                for j in range(NCH):
                    vtmp = scr.tile([128, F], DT, name="vtmp", tag="vtmp")
                    nc.vector.scalar_tensor_tensor(vtmp[:, :], Wa[j][:, :], -0.25,
                                                   o_sb[j][:, :], Op.mult, Op.add)
                    nc.scalar.activation(Ea[j][:, :], vtmp[:, :], SIGN, bias=bias_nh[:, 0:1])
                    reset(vtmp, Ea[j], Wa[j])

                # ---- proj conv + LIF (sign-coded) ----
                Ep = [sp.tile([128, F], DT, name=f"Ep{j}", tag=f"Ep{j}") for j in range(NCH)]
                for co in range(NCH):
                    P = ps.tile([128, F], DT, name="Pcv", tag="Pcv")
                    conv("p", Ea, NCH, co, P)
                    vtmp = scr.tile([128, F], DT, name="vtmp", tag="vtmp")
                    lif_add(P, Wpj[co], vtmp, -0.25)
                    nc.scalar.activation(Ep[co][:, :], vtmp[:, :], SIGN, bias=bias_n1[:, 0:1])
                    reset(vtmp, Ep[co], Wpj[co])

                # ---- residual 1 (in place): xp1 <- (x+1) + 0.5*Ep = xr + 0.5 ----
                for j in range(NCH):
                    nc.vector.scalar_tensor_tensor(xp1[j][:, :], Ep[j][:, :], 0.5,
                                                   xp1[j][:, :], Op.mult, Op.add)

                # ---- fc1 + LIF ----
                E1 = [sp.tile([128, F], DT, name=f"E1{j}",
                              tag=(f"Sq{j}" if j < 3 else f"Sk{j-3}" if j < 6
                                   else f"Sv{j-6}" if j < 9 else f"Ea{j-9}"))
                      for j in range(NH1)]
                for co in range(NH1):
                    P = ps.tile([128, F], DT, name="Pcv", tag="Pcv")
                    conv("1", xp1, NCH, co, P)
                    vtmp = scr.tile([128, F], DT, name="vtmp", tag="vtmp")
                    lif_add(P, W1[co], vtmp, -0.25)
                    nc.scalar.activation(E1[co][:, :], vtmp[:, :], SIGN, bias=bias_n1[:, 0:1])
                    reset(vtmp, E1[co], W1[co])

                # ---- fc2 + LIF + residual 2 + store ----
                for co in range(NCH):
                    P = ps.tile([128, F], DT, name="Pcv", tag="Pcv")
                    conv("2", E1, NH1, co, P)
                    vtmp = scr.tile([128, F], DT, name="vtmp", tag="vtmp")
                    lif_add(P, W2[co], vtmp, -0.25)
                    e2 = scr.tile([128, F], DT, name="e2", tag="e2")
                    nc.scalar.activation(e2[:, :], vtmp[:, :], SIGN, bias=bias_n1[:, 0:1])
                    reset(vtmp, e2, W2[co])
                    ot = scr.tile([128, F], DT, name="ot", tag="ot")
                    nc.vector.scalar_tensor_tensor(ot[:, :], e2[:, :], 0.5,
                                                   xp1[co][:, :], Op.mult, Op.add)
                    dma(out_d[t, co, :, bo:bo + NBP, :].rearrange("p b n -> p (b n)"), ot[:])

    _split_multiwaits(nc)
    return nc


def kernel(**inputs):
    if "nc" not in _CACHED:
        _CACHED["nc"] = _build_program()
    nc = _CACHED["nc"]
    in_maps = _prep_host(inputs)
    res = run_bass_kernel_spmd(nc, in_maps, list(range(NCORES)))
    out = np.empty((T, B, C, N), dtype=np.float32)
    for c in range(NCORES):
        o = np.asarray(res.results[c]["out"])  # [T, NCH, 128, NB, N]
        out[:, c * NB:(c + 1) * NB] = o.transpose(0, 3, 1, 2, 4).reshape(T, NB, C, N)
    return out


# revision 16
# speedup vs baseline: 17407.8569x; 17407.8569x over previous
"""Spikformer-style block (conv1x1+BN+LIF, policy-masked spiking attention, MLP)
on 8 Trainium2 NeuronCores, data-parallel over batch.

Each core handles 4 batches, processed as 2 passes of 2 batches (SBUF budget).
Layout: activations [C-chunk(128), (b, n)] with 2 batches, n=196 -> F=392.
All arithmetic fp32. Attention uses the exact decomposition
  o[n] = q[n] @ (k^T diag(pr) v) + (1-pr[n]) (q[n].k[n]) v[n]
which is integer-exact for binary spikes/policy.
LIF state is stored negated (W = -v_reset, or -2*v_reset for sign-coded layers)
so each step is: v = (W * leak) + psum ; spike ; W' = (spike - 1) * v.
"""

from contextlib import ExitStack

import numpy as np

import concourse.bass as bass
from concourse import mybir, tile
from concourse.mybir import AluOpType as Op
from concourse.bass_utils import run_bass_kernel_spmd

T, B, C, N, H, D, HID = 4, 32, 384, 196, 12, 32, 1536
EPS = 1e-5
NCORES = 8
NB = B // NCORES          # batches per core (4)
NBP = 2                   # batches per pass
NPASS = NB // NBP
F = NBP * N               # 392 free elements per tile row
NCH = C // 128            # 3 channel chunks
NH1 = HID // 128          # 12 hidden chunks
DT = mybir.dt.float32
DTB = mybir.dt.bfloat16

_CACHED = {}


def _split_multiwaits(nc):
    """Hardware TPB instructions hold one sync wait; hoist extras onto
    injected same-engine NoOps placed immediately before."""
    ctr = 0
    for f in nc.m.functions:
        for blk in f.blocks:
            insts = blk.instructions
            new = []
            changed = False
            for inst in insts:
                si = inst.sync_info
                if si is not None and si.on_wait and len(si.on_wait) > 1:
                    waits = list(si.on_wait)
                    for w in waits[:-1]:
                        ctr += 1
                        nop = mybir.InstNoOp(name=f"I-wsplit{ctr}")
                        nop.engine = inst.engine
                        nop.sync_info = mybir.SyncInfo(on_wait=[w], on_update=[])
                        new.append(nop)
                    inst.sync_info = mybir.SyncInfo(
                        on_wait=[waits[-1]], on_update=list(si.on_update or []))
                    changed = True
                new.append(inst)
            if changed:
                blk.instructions = new


def _fold(w, b, g, beta, m, var):
    """0.5*conv_bn(x) = W_eff @ x + d."""
    inv = g / np.sqrt(var + EPS)
    shift = (b if b is not None else 0.0) * inv + beta - m * inv
    return 0.5 * inv[:, None] * w, 0.5 * shift


def _prep_host(inputs):
    f32 = lambda a: np.ascontiguousarray(a, dtype=np.float32)
    x = np.asarray(inputs["x"], dtype=np.float32)
    pol = np.asarray(inputs["policy"], dtype=np.float32).reshape(T, B, N)

    wq, dq = _fold(inputs["wq"], None, inputs["qg"], inputs["qb"], inputs["qm"], inputs["qv"])
    wk, dk = _fold(inputs["wk"], None, inputs["kg"], inputs["kb"], inputs["km"], inputs["kvv"])
    wv, dv = _fold(inputs["wv"], None, inputs["vg"], inputs["vb"], inputs["vm"], inputs["vvv"])
    wp, dp = _fold(inputs["wp"], inputs["bp"], inputs["pg"], inputs["pb"], inputs["pm"], inputs["pv"])
    w1, d1 = _fold(inputs["w1"], inputs["b1"], inputs["g1"], inputs["be1"], inputs["m1"], inputs["v1"])
    w2, d2 = _fold(inputs["w2"], inputs["b2"], inputs["g2"], inputs["be2"], inputs["m2"], inputs["v2"])

    # encodings: q/k/v consume x+1; proj/fc2 consume +-1 spikes; fc1 consumes xr+0.5
    dq = dq - wq.sum(1); dk = dk - wk.sum(1); dv = dv - wv.sum(1)
    dp = dp + 0.5 * wp.sum(1); wp = 0.5 * wp
    d1 = d1 - 0.5 * w1.sum(1)
    d2 = d2 + 0.5 * w2.sum(1); w2 = 0.5 * w2

    def wT(w):  # [O,I] -> [I//128, 128, O]
        I = w.shape[1]
        return f32(w.T.reshape(I // 128, 128, -1))

    # pack big [128, *] constants into one wall, [1, *] rows into another
    wallw = np.concatenate(
        [np.concatenate(list(wT(w)), axis=1) for w in (wq, wk, wv, wp, w1, w2)]
        + [np.eye(128, dtype=np.float32),
           np.kron(np.eye(4), np.ones((32, 32))).astype(np.float32)], axis=1)
    walld = np.concatenate(
        [dq[None], dk[None], dv[None], dp[None], d1[None], d2[None],
         np.ones((1, F), np.float32), np.ones((1, 128), np.float32)], axis=1)
    com = {"wallw": f32(wallw), "walld": f32(walld)}

    xp1 = f32((x + 1.0).reshape(T, B, NCH, 128, N).transpose(0, 2, 3, 1, 4))
    prm1 = f32(1.0 - pol)

    in_maps = []
    for c in range(NCORES):
        bs = slice(c * NB, (c + 1) * NB)
        m = dict(com)
        m["xp1"] = f32(xp1[:, :, :, bs, :])
        m["prcol"] = f32(pol[:, bs, :].reshape(T, NB, N, 1))
        m["prm1row"] = f32(prm1[:, bs, :].reshape(T, 1, NB * N))
        in_maps.append(m)
    return in_maps


def _build_program(reps=1):
    nc = bass.Bass("TRN2")
    p = lambda name, shape: nc.declare_dram_parameter(name, list(shape), DT, isOutput=False)

    xp1_d = p("xp1", (T, NCH, 128, NB, N))
    WTW = 3 * C + 3 * C + 3 * C + 3 * C + 3 * HID + 12 * C + 256
    WTD = 4 * C + HID + C + F + 128
    wallw_d = p("wallw", (128, WTW))
    walld_d = p("walld", (1, WTD))
    prcol_d = p("prcol", (T, NB, N, 1))
    prm1_d = p("prm1row", (T, 1, NB * N))
    out_d = nc.declare_dram_parameter("out", [T, NCH, 128, NB, N], DT, isOutput=True)

    SIGN = mybir.ActivationFunctionType.Sign
    COPY = mybir.ActivationFunctionType.Copy

    with tile.TileContext(nc) as tc, ExitStack() as ctx:
        cst = ctx.enter_context(tc.tile_pool(name="cst", bufs=1))
        st = ctx.enter_context(tc.tile_pool(name="state", bufs=1))
        act = ctx.enter_context(tc.tile_pool(name="act", bufs=2))
        sp = ctx.enter_context(tc.tile_pool(name="spk", bufs=1))
        scr = ctx.enter_context(tc.tile_pool(name="scr", bufs=2))
        ps = ctx.enter_context(tc.tile_pool(name="ps", bufs=2, space="PSUM"))

        dma = nc.gpsimd.dma_start

        # ---- constants / weights: two wall DMAs, sliced views ----
        wallw = cst.tile([128, WTW], DT, name="wallw", tag="wallw")
        dma(wallw[:], wallw_d[:])
        walld = cst.tile([1, WTD], DT, name="walld", tag="walld")
        dma(walld[:], walld_d[:])
        bias_nh = cst.tile([128, 1], DT, name="bias_nh", tag="bias_nh")
        nc.vector.memset(bias_nh[:], -0.5)
        bias_n1 = cst.tile([128, 1], DT, name="bias_n1", tag="bias_n1")
        nc.vector.memset(bias_n1[:], -1.0)
        wsb, dsb = {}, {}
        off = 0
        for L, nci in (("q", NCH), ("k", NCH), ("v", NCH), ("p", NCH), ("1", NCH), ("2", NH1)):
            O = HID if L == "1" else C
            wsb[L] = []
            for ci in range(nci):
                wsb[L].append(wallw[:, off:off + O])
                off += O
        ident = wallw[:, off:off + 128]; off += 128
        bones = wallw[:, off:off + 128]; off += 128
        doff = 0
        for L in ("q", "k", "v", "p", "1", "2"):
            O = HID if L == "1" else C
            dsb[L] = walld[:, doff:doff + O]
            doff += O
        onesr = walld[:, doff:doff + F]; doff += F
        onesc = walld[:, doff:doff + 128]; doff += 128
        identb = cst.tile([128, 128], DTB, name="identb", tag="identb")
        nc.scalar.copy(identb[:], ident[:, :])
        bonesb = cst.tile([128, 128], DTB, name="bonesb", tag="bonesb")
        nc.scalar.copy(bonesb[:], bones[:, :])

        # ---- persistent LIF states (re-zeroed per pass) ----
        def states(tag, n):
            return [st.tile([128, F], DT, name=f"{tag}{i}", tag=f"{tag}{i}") for i in range(n)]

        MSB = [[st.tile([128, 128], DTB, name=f"MSB{r}{j}", tag=f"MSB{r}{j}")
                for j in range(NCH)] for r in range(2)]
        for r in range(2):
            for j in range(NCH):
                nc.vector.memset(MSB[r][j][:], 0.0)
        Wq, Wk, Wv = states("Wq", NCH), states("Wk", NCH), states("Wv", NCH)
        Wa, Wpj = states("Wa", NCH), states("Wp", NCH)
        W1, W2 = states("W1", NH1), states("W2", NCH)
        ALL_W = Wq + Wk + Wv + Wa + Wpj + W1 + W2

        def conv(wL, rhs_tiles, nci, co, P):
            for ci in range(nci):
                nc.tensor.matmul(
                    P[:, 0:F], lhsT=wsb[wL][ci][:, co * 128:(co + 1) * 128],
                    rhs=rhs_tiles[ci][:, 0:F], start=(ci == 0), stop=False)
            nc.tensor.matmul(
                P[:, 0:F], lhsT=dsb[wL][0:1, co * 128:(co + 1) * 128],
                rhs=onesr[0:1, 0:F], start=False, stop=True)

        def lif_add(P, Wt, vtmp, leak):
            nc.vector.scalar_tensor_tensor(
                vtmp[:, :], Wt[:, :], leak, P[:, 0:F], Op.mult, Op.add)

        def reset(vtmp, S, Wt):
            nc.vector.scalar_tensor_tensor(
                Wt[:, :], S[:, :], 1.0, vtmp[:, :], Op.subtract, Op.mult)

        # =========================== pass / time loops ===========================
        for pp in range(NPASS * reps):
            pp = pp % NPASS
            bo = pp * NBP  # batch offset within this core's 4
            for Wt in ALL_W:
                nc.vector.memset(Wt[:], 0.0)

            for t in range(T):
                xp1 = [act.tile([128, F], DT, name=f"xp1{j}", tag=f"xp1{j}") for j in range(NCH)]
                for j in range(NCH):
                    dma(xp1[j][:], xp1_d[t, j, :, bo:bo + NBP, :].rearrange("p b n -> p (b n)"))

                # ---- q,k,v convs + LIF ({0,1} spikes) ----
                Sq = [sp.tile([128, F], DTB, name=f"Sq{j}", tag=f"Sq{j}") for j in range(NCH)]
                Sk = [sp.tile([128, F], DTB, name=f"Sk{j}", tag=f"Sk{j}") for j in range(NCH)]
                Sv = [sp.tile([128, F], DTB, name=f"Sv{j}", tag=f"Sv{j}") for j in range(NCH)]
                for L, Wx, Sx in (("q", Wq, Sq), ("k", Wk, Sk), ("v", Wv, Sv)):
                    for co in range(NCH):
                        P = ps.tile([128, F], DT, name="Pcv", tag="Pcv")
                        conv(L, xp1, NCH, co, P)
                        vtmp = scr.tile([128, F], DT, name="vtmp", tag="vtmp")
                        lif_add(P, Wx[co], vtmp, -0.5)
                        nc.vector.tensor_scalar(Sx[co][:, :], vtmp[:, :], 1.0, None, Op.is_ge)
                        if t < T - 1:
                            reset(vtmp, Sx[co], Wx[co])

                # ---- attention ----
                prm1_r = scr.tile([1, F], DT, name="prm1r", tag="prm1r", bufs=1)
                dma(prm1_r[:], prm1_d[t, :, bo * N:(bo + NBP) * N])
                Pb = ps.tile([128, F], DT, name="Pcv", tag="Pcv")
                nc.tensor.matmul(Pb[:, 0:F], lhsT=onesc[0:1, :], rhs=prm1_r[0:1, 0:F],
                                 start=True, stop=True)
                prm1_bc = scr.tile([128, F], DTB, name="prm1bc", tag="prm1bc", bufs=1)
                nc.scalar.activation(prm1_bc[:, :], Pb[:, 0:F], COPY)

                o_sb = [scr.tile([128, F], DT, name=f"osb{j}", tag=f"osb{j}", bufs=1)
                        for j in range(NCH)]
                for b in range(NBP):
                    # transposes of Sk, Sv -> [n, c] layout; policy folded into v
                    KT, VT = [], []
                    for ns in range(2):
                        w_ = 128 if ns == 0 else N - 128
                        pc = scr.tile([w_, 1], DT, name=f"prc{ns}", tag=f"prc{ns}")
                        dma(pc[:], prcol_d[t, bo + b, ns * 128: ns * 128 + w_])
                        kt = scr.tile([w_, C], DTB, name=f"KT{ns}", tag=f"KT{ns}")
                        vt = scr.tile([w_, C], DTB, name=f"VT{ns}", tag=f"VT{ns}")
                        for src, dst, scale in ((Sk, kt, None), (Sv, vt, pc)):
                            for j in range(NCH):
                                Pt = ps.tile([128, 128], DTB, name="Ptp", tag="Ptp")
                                nc.tensor.transpose(
                                    Pt[0:w_, 0:128],
                                    src[j][:, b * N + ns * 128: b * N + ns * 128 + w_],
                                    identb[:, :])
                                if scale is None:
                                    nc.scalar.activation(dst[:, j * 128:(j + 1) * 128],
                                                         Pt[0:w_, 0:128], COPY)
                                else:
                                    nc.scalar.activation(dst[:, j * 128:(j + 1) * 128],
                                                         Pt[0:w_, 0:128], COPY,
                                                         scale=scale[:, 0:1])
                        KT.append(kt); VT.append(vt)
                    # per-chunk cross-head Gram; keep only diagonal 32x32 blocks
                    # (= per-head M, 0.125 folded on evict into pre-zeroed
                    # block-diagonal tiles), then one full-K matmul applies it.
                    for j in range(NCH):
                        Pg = ps.tile([128, 128], DT, name="Pg", tag="Ptp")
                        for ns in range(2):
                            nc.tensor.matmul(
                                Pg[:, 0:128],
                                lhsT=KT[ns][:, j * 128:(j + 1) * 128],
                                rhs=VT[ns][:, j * 128:(j + 1) * 128],
                                start=(ns == 0), stop=(ns == 1))
                        mb = MSB[b % 2][j]
                        for hl in range(4):
                            nc.scalar.activation(
                                mb[32 * hl:32 * hl + 32, 32 * hl:32 * hl + 32],
                                Pg[32 * hl:32 * hl + 32, 32 * hl:32 * hl + 32],
                                COPY, scale=0.125)
                        Po = ps.tile([128, N], DT, name="Po", tag="Po")
                        nc.tensor.matmul(Po[:, 0:N], lhsT=mb[:, :],
                                         rhs=Sq[j][:, b * N:(b + 1) * N],
                                         start=True, stop=True)
                        nc.scalar.activation(o_sb[j][:, b * N:(b + 1) * N], Po[:, 0:N], COPY)

                # diagonal correction: o += 0.125 * (1-pr) * (q.k)_head * v
                for j in range(NCH):
                    qk = scr.tile([128, F], DTB, name="qk", tag="qk", bufs=1)
                    nc.vector.tensor_tensor(qk[:, :], Sq[j][:, :], Sk[j][:, :], Op.mult)
                    Pc = ps.tile([128, F], DT, name="Pcv", tag="Pcv")
                    nc.tensor.matmul(Pc[:, 0:F], lhsT=bonesb[:, :], rhs=qk[:, 0:F],
                                     start=True, stop=True)
                    wti = scr.tile([128, F], DTB, name="wti", tag="wti", bufs=1)
                    nc.vector.tensor_tensor(wti[:, :], Pc[:, 0:F], prm1_bc[:, :], Op.mult)
                    u = scr.tile([128, F], DTB, name="u", tag="u", bufs=1)
                    nc.vector.scalar_tensor_tensor(u[:, :], wti[:, :], 0.125, Sv[j][:, :],
                                                   Op.mult, Op.mult)
                    nc.vector.tensor_tensor(o_sb[j][:, :], o_sb[j][:, :], u[:, :], Op.add)

                # ---- attn LIF (thr 0.5, sign-coded) ----
                Ea = [sp.tile([128, F], DT, name=f"Ea{j}", tag=f"Ea{j}") for j in range(NCH)]
                for j in range(NCH):
                    vtmp = scr.tile([128, F], DT, name="vtmp", tag="vtmp")
                    nc.vector.scalar_tensor_tensor(vtmp[:, :], Wa[j][:, :], -0.25,
                                                   o_sb[j][:, :], Op.mult, Op.add)
                    nc.scalar.activation(Ea[j][:, :], vtmp[:, :], SIGN, bias=bias_nh[:, 0:1])
                    if t < T - 1:
                        reset(vtmp, Ea[j], Wa[j])

                # ---- proj conv + LIF (sign-coded) ----
                Ep = [sp.tile([128, F], DTB, name=f"Ep{j}", tag=f"Ep{j}") for j in range(NCH)]
                for co in range(NCH):
                    P = ps.tile([128, F], DT, name="Pcv", tag="Pcv")
                    conv("p", Ea, NCH, co, P)
                    vtmp = scr.tile([128, F], DT, name="vtmp", tag="vtmp")
                    lif_add(P, Wpj[co], vtmp, -0.25)
                    nc.scalar.activation(Ep[co][:, :], vtmp[:, :], SIGN, bias=bias_n1[:, 0:1])
                    if t < T - 1:
                        reset(vtmp, Ep[co], Wpj[co])

                # ---- residual 1 (in place): xp1 <- (x+1) + 0.5*Ep = xr + 0.5 ----
                for j in range(NCH):
                    nc.vector.scalar_tensor_tensor(xp1[j][:, :], Ep[j][:, :], 0.5,
                                                   xp1[j][:, :], Op.mult, Op.add)

                # ---- fc1 + LIF ----
                E1 = [sp.tile([128, F], DT, name=f"E1{j}",
                              tag=(f"Sq{j}" if j < 3 else f"Sk{j-3}" if j < 6
                                   else f"Sv{j-6}" if j < 9 else f"Ea{j-9}"))
                      for j in range(NH1)]
                for co in range(NH1):
                    P = ps.tile([128, F], DT, name="Pcv", tag="Pcv")
                    conv("1", xp1, NCH, co, P)
                    vtmp = scr.tile([128, F], DT, name="vtmp", tag="vtmp")
                    lif_add(P, W1[co], vtmp, -0.25)
                    nc.scalar.activation(E1[co][:, :], vtmp[:, :], SIGN, bias=bias_n1[:, 0:1])
                    if t < T - 1:
                        reset(vtmp, E1[co], W1[co])

                # ---- fc2 + LIF + residual 2 + store ----
                for co in range(NCH):
                    P = ps.tile([128, F], DT, name="Pcv", tag="Pcv")
                    conv("2", E1, NH1, co, P)
                    vtmp = scr.tile([128, F], DT, name="vtmp", tag="vtmp")
                    lif_add(P, W2[co], vtmp, -0.25)
                    e2 = scr.tile([128, F], DTB, name="e2", tag="e2")
                    nc.scalar.activation(e2[:, :], vtmp[:, :], SIGN, bias=bias_n1[:, 0:1])
                    if t < T - 1:
                        reset(vtmp, e2, W2[co])
                    ot = scr.tile([128, F], DT, name="ot", tag="ot")
                    nc.vector.scalar_tensor_tensor(ot[:, :], e2[:, :], 0.5,
                                                   xp1[co][:, :], Op.mult, Op.add)
                    dma(out_d[t, co, :, bo:bo + NBP, :].rearrange("p b n -> p (b n)"), ot[:])

    _split_multiwaits(nc)
    return nc


def kernel(**inputs):
    if "nc" not in _CACHED:
        _CACHED["nc"] = _build_program()
    nc = _CACHED["nc"]
    in_maps = _prep_host(inputs)
    res = run_bass_kernel_spmd(nc, in_maps, list(range(NCORES)))
    out = np.empty((T, B, C, N), dtype=np.float32)
    for c in range(NCORES):
        o = np.asarray(res.results[c]["out"])  # [T, NCH, 128, NB, N]
        out[:, c * NB:(c + 1) * NB] = o.transpose(0, 3, 1, 2, 4).reshape(T, NB, C, N)
    return out
